# revision 16
# baseline (speedup 1.0000x reference)
"""ALCOVE cell Bass kernel for 8 TRN2 NeuronCores (data-parallel over batch).

B=32, T=16, N_RBF=1024, N_DIM=64, UNITS=64. 4 batches per core.

State per batch: attention (64,), association (1024, 64).
Layout: R=1024 on partitions as 8 chunks of 128; per-batch row data
(att, x, dx, g_att) on partition 0 as (1, B_LOC*64) rows (PE base-partition
rule); partition broadcasts via K=1 ones-matmul.
"""

import numpy as np

B, T, R, D, U = 32, 16, 1024, 64, 64
NCHUNK, P = 8, 128
EPS = 1e-6
SQEPS = 1e-12
N_CORES = 8
B_LOC = B // N_CORES  # 4

_cache = {}


def _build(rho, temperature, lr_att, lr_assoc, beta):
    import concourse.bass as bass
    import concourse.tile as tile
    from concourse import bacc, mybir

    f32 = mybir.dt.float32
    AF = mybir.ActivationFunctionType
    OP = mybir.AluOpType

    nc = bacc.Bacc("TRN2", target_bir_lowering=False, debug=False, num_devices=N_CORES)
    # single packed input: [embedB (512) | zbcast (T*B*D=4096) | oh row (T*B*U=1024)]
    FIN = NCHUNK * D + T * B_LOC * D + T * B_LOC * U
    big_in = nc.declare_dram_parameter("big", [P, FIN], f32, isOutput=False)
    out_ext = nc.declare_dram_parameter("out", [B_LOC, T * U], f32, isOutput=True)

    with tile.TileContext(nc) as tc:
        with (
            tc.tile_pool(name="persist", bufs=1) as persist,
            tc.tile_pool(name="work", bufs=3) as work,
            tc.tile_pool(name="psum", bufs=2, space="PSUM") as psum,
            tc.tile_pool(name="psmall", bufs=2, space="PSUM") as psmall,
        ):
            # ---- persistent tiles (one DMA for all inputs) ----
            big = persist.tile([P, FIN], f32)
            nc.gpsimd.dma_start(big[:], big_in[:])
            embedB = big[:, 0 : NCHUNK * D]
            zb = big[:, NCHUNK * D : NCHUNK * D + T * B_LOC * D].rearrange("p (t f) -> p t f", t=T)
            oh = big[0:1, NCHUNK * D + T * B_LOC * D :].rearrange("p (t f) -> p t f", t=T)

            ones_row = persist.tile([1, P], f32)
            nc.vector.memset(ones_row[:], 1.0)
            consts = persist.tile([P, 3], f32)
            nc.vector.memset(consts[:, 0:1], 0.0)
            nc.vector.memset(consts[:, 1:2], 1.0)
            nc.vector.memset(consts[:, 2:3], EPS)
            czero, cone, ceps = consts[:, 0:1], consts[:, 1:2], consts[:, 2:3]

            att = persist.tile([1, B_LOC * D], f32)  # row form, state
            nc.vector.memset(att[:], 1.0 / D)
            assoc = [persist.tile([P, NCHUNK, U], f32, name=f"assoc{b}") for b in range(B_LOC)]
            for b in range(B_LOC):
                nc.vector.memset(assoc[b][:], 0.0)

            xs4 = persist.tile([B_LOC, T, U], f32)
            probs = persist.tile([B_LOC, T, U], f32)
            sume = persist.tile([B_LOC, T], f32)

            for t in range(T):
                # -------- attention broadcast: (1, B*D) -> psum (P, B_LOC, D)
                attb_ps = psum.tile([P, B_LOC, D], f32, tag="attb")
                nc.tensor.matmul(attb_ps[:, :, :].rearrange("p b d -> p (b d)"),
                                 ones_row[:], att[0:1, :], start=True, stop=True)
                attb = work.tile([P, B_LOC, D], f32, tag="attb_sb")
                nc.scalar.copy(attb[:], attb_ps[:])

                dpow_l = []
                qall = work.tile([P, B_LOC, NCHUNK], f32, tag="qall")
                for b in range(B_LOC):
                    # -------- diff^rho chain, (P, 512)
                    diff = work.tile([P, NCHUNK, D], f32, tag=f"diff{b}", name=f"diff{b}")
                    zrep = zb[:, t, b * D : (b + 1) * D][:, None, :].broadcast_to([P, NCHUNK, D])
                    nc.vector.tensor_tensor(diff[:], embedB.rearrange("p (c d) -> p c d", d=D), zrep, op=OP.subtract)
                    nc.scalar.activation(diff[:], diff[:], AF.Abs, bias=czero)
                    nc.scalar.activation(diff[:], diff[:], AF.Ln, bias=ceps)
                    dpow = work.tile([P, NCHUNK, D], f32, tag=f"dpow{b}", name=f"dpow{b}")
                    nc.scalar.activation(dpow[:], diff[:], AF.Exp, bias=czero, scale=rho)
                    dpow_l.append(dpow)
                    # -------- q: per-chunk fused mult+reduce
                    qjunk = work.tile([P, D], f32, tag="qjunk", name="qjunk")
                    for c in range(NCHUNK):
                        nc.vector.scalar_tensor_tensor(
                            qjunk[:], dpow[:, c, :], 1.0, attb[:, b, :],
                            op0=OP.mult, op1=OP.mult,
                            accum_out=qall[:, b, c : c + 1])

                # -------- similarity acts on (P, B_LOC*NCHUNK)
                lnq = work.tile([P, B_LOC, NCHUNK], f32, tag="lnq")
                nc.scalar.activation(lnq[:], qall[:], AF.Ln, bias=ceps)
                s_sim = work.tile([P, B_LOC, NCHUNK], f32, tag="s_sim")
                nc.scalar.activation(s_sim[:], lnq[:], AF.Exp, bias=czero, scale=1.0 / rho)  # = d
                nc.scalar.activation(s_sim[:], s_sim[:], AF.Exp, bias=czero, scale=-beta)    # = s
                qp = work.tile([P, B_LOC, NCHUNK], f32, tag="qp")
                nc.scalar.activation(qp[:], lnq[:], AF.Exp, bias=czero, scale=(1.0 - rho) / rho)

                # -------- x: per batch 8 accumulating matmuls, M=1 row 0
                x_ps = psmall.tile([1, B_LOC * U], f32, tag="x_ps")
                for b in range(B_LOC):
                    for c in range(NCHUNK):
                        nc.tensor.matmul(x_ps[0:1, b * U : (b + 1) * U],
                                         s_sim[:, b, c : c + 1],
                                         assoc[b][:, c, :],
                                         start=(c == 0), stop=(c == NCHUNK - 1))

                # -------- teacher / dx on (1, B_LOC*U)
                pp = work.tile([1, B_LOC * U], f32, tag="pp")
                nc.scalar.activation(pp[:], x_ps[:], AF.Relu, bias=cone[:1, :])          # relu(x+1)
                mrow = work.tile([1, B_LOC * U], f32, tag="mrow")
                nc.scalar.activation(mrow[:], x_ps[:], AF.Relu, bias=cone[:1, :], scale=-1.0)  # relu(1-x)
                # save x for softmax, scattered to (B_LOC, U)
                xrow = work.tile([1, B_LOC * U], f32, tag="xrow")
                nc.scalar.copy(xrow[:], x_ps[:])
                for b in range(B_LOC):
                    nc.gpsimd.dma_start(xs4[b : b + 1, t, :],
                                        xrow[0:1, b * U : (b + 1) * U])
                nc.vector.tensor_tensor(mrow[:], pp[:], mrow[:], op=OP.add)          # p+m
                nc.vector.tensor_tensor(mrow[:], mrow[:], oh[0:1, t, :], op=OP.mult) # oh*(p+m)
                dxrow = work.tile([1, B_LOC * U], f32, tag="dxrow")
                nc.vector.tensor_tensor(dxrow[:], pp[:], mrow[:], op=OP.subtract)

                # -------- dx broadcast to (P, B_LOC, U)
                dxb_ps = psum.tile([P, B_LOC, U], f32, tag="dxb")
                nc.tensor.matmul(dxb_ps[:, :, :].rearrange("p b d -> p (b d)"),
                                 ones_row[:], dxrow[0:1, :], start=True, stop=True)
                dxb = work.tile([P, B_LOC, U], f32, tag="dxb_sb")
                nc.scalar.copy(dxb[:], dxb_ps[:])

                # -------- y + c
                yall = work.tile([P, B_LOC, NCHUNK], f32, tag="yall")
                yjunk = work.tile([P, U], f32, tag="yjunk", name="yjunk")
                for b in range(B_LOC):
                    for c in range(NCHUNK):
                        nc.vector.scalar_tensor_tensor(
                            yjunk[:], assoc[b][:, c, :], 1.0, dxb[:, b, :],
                            op0=OP.mult, op1=OP.mult,
                            accum_out=yall[:, b, c : c + 1])
                call = work.tile([P, B_LOC, NCHUNK], f32, tag="call")
                nc.vector.tensor_tensor(call[:], s_sim[:], qp[:], op=OP.mult)
                nc.vector.scalar_tensor_tensor(call[:], yall[:], -beta / rho, call[:],
                                               op0=OP.mult, op1=OP.mult)

                # -------- g_att (PE, M=1 row 0) + att update
                gatt_ps = psmall.tile([1, B_LOC * D], f32, tag="gatt")
                for b in range(B_LOC):
                    for c in range(NCHUNK):
                        nc.tensor.matmul(gatt_ps[0:1, b * D : (b + 1) * D],
                                         call[:, b, c : c + 1],
                                         dpow_l[b][:, c, :],
                                         start=(c == 0), stop=(c == NCHUNK - 1))
                nc.vector.scalar_tensor_tensor(att[:], gatt_ps[:], -lr_att, att[:],
                                               op0=OP.mult, op1=OP.add)
                nc.vector.tensor_scalar_max(att[:], att[:], 0.0)

                # -------- assoc update: assoc += (-lr*s_rc) * dx_bu
                slr = work.tile([P, B_LOC, NCHUNK], f32, tag="slr")
                nc.vector.tensor_scalar_mul(slr[:], s_sim[:], -lr_assoc)
                for b in range(B_LOC):
                    for c in range(NCHUNK):
                        nc.vector.scalar_tensor_tensor(
                            assoc[b][:, c, :], dxb[:, b, :], slr[:, b, c : c + 1],
                            assoc[b][:, c, :], op0=OP.mult, op1=OP.add)

                # -------- softmax(temp*x) for this t (stable), (B_LOC, U)
                mx = work.tile([B_LOC, 1], f32, tag="mx")
                nc.vector.tensor_reduce(mx[:], xs4[:, t, :], axis=mybir.AxisListType.X, op=OP.max)
                nc.vector.tensor_scalar_mul(mx[:], mx[:], -temperature)
                nc.scalar.activation(probs[:, t, :], xs4[:, t, :], AF.Exp,
                                     bias=mx[:], scale=temperature,
                                     accum_out=sume[:, t : t + 1])

            # -------- normalize + store
            rec = work.tile([B_LOC, T], f32, tag="rec")
            nc.vector.reciprocal(rec[:], sume[:])
            for t in range(T):
                nc.vector.tensor_scalar_mul(probs[:, t, :], probs[:, t, :], rec[:, t : t + 1])
            nc.gpsimd.dma_start(out_ext[:], probs[:, :, :].rearrange("b t u -> b (t u)"))

    nc.compile()
    return nc


def kernel(stimulus_set, label_idx, embed, rho, temperature, lr_attention, lr_association, beta):
    from concourse.bass_utils import run_bass_kernel_spmd

    stimulus_set = np.asarray(stimulus_set)
    label_idx = np.asarray(label_idx)
    embed = np.asarray(embed, dtype=np.float32)
    rho_f = float(rho)
    temp_f = float(temperature)
    lra_f = float(lr_attention)
    lrs_f = float(lr_association)
    beta_f = float(beta)

    key = (rho_f, temp_f, lra_f, lrs_f, beta_f)
    if key not in _cache:
        _cache[key] = _build(rho_f, temp_f, lra_f, lrs_f, beta_f)
    nc = _cache[key]

    # host-side prep
    embedB = embed.reshape(NCHUNK, P, D).transpose(1, 0, 2).reshape(P, NCHUNK * D).copy()
    z = embed[stimulus_set]  # (B, T, D)
    onehot = np.zeros((B, T, U), dtype=np.float32)
    bi, ti = np.meshgrid(np.arange(B), np.arange(T), indexing="ij")
    onehot[bi, ti, label_idx] = 1.0

    in_maps = []
    for i in range(N_CORES):
        bs = slice(i * B_LOC, (i + 1) * B_LOC)
        zc = z[bs].transpose(1, 0, 2).reshape(1, T * B_LOC * D)
        zbcast = np.broadcast_to(zc, (P, T * B_LOC * D)).copy()
        ohrow = onehot[bs].transpose(1, 0, 2).reshape(1, T * B_LOC * U)
        ohfull = np.broadcast_to(ohrow, (P, T * B_LOC * U))
        big = np.concatenate([embedB, zbcast, ohfull], axis=1).astype(np.float32)
        in_maps.append({"big": np.ascontiguousarray(big)})

    res = run_bass_kernel_spmd(nc, in_maps, core_ids=list(range(N_CORES)))
    outs = [res.results[i]["out"].reshape(B_LOC, T, U) for i in range(N_CORES)]
    return np.concatenate(outs, axis=0)


def _install_ntff_hook():
    import sys, types, ctypes, contextlib
    if "antenv.axon_hooks" in sys.modules:
        return
    import antenv
    mod = types.ModuleType("antenv.axon_hooks")
    mod._hook = None
    def set_axon_ntff_profile_hook(h):
        mod._hook = h
    def get_axon_ntff_profile_hook():
        return mod._hook
    mod.set_axon_ntff_profile_hook = set_axon_ntff_profile_hook
    mod.get_axon_ntff_profile_hook = get_axon_ntff_profile_hook
    sys.modules["antenv.axon_hooks"] = mod
    antenv.axon_hooks = mod

    so_path = "/opt/axon/libaxon_pjrt.so"
    lib = ctypes.CDLL(so_path)
    if not hasattr(lib, "axon_start_nrt_profile"):
        return
    lib.axon_start_nrt_profile.argtypes = [ctypes.POINTER(ctypes.c_int64), ctypes.c_size_t]
    lib.axon_start_nrt_profile.restype = ctypes.c_int64
    lib.axon_stop_nrt_profile.argtypes = [ctypes.c_char_p]
    lib.axon_stop_nrt_profile.restype = ctypes.c_int64

    @contextlib.contextmanager
    def _hook(output_dir, device_ids):
        import jax
        jax.devices()
        if device_ids:
            ids = (ctypes.c_int64 * len(device_ids))(*device_ids)
            rc = lib.axon_start_nrt_profile(ids, len(device_ids))
        else:
            rc = lib.axon_start_nrt_profile(None, 0)
        if rc != 0:
            raise RuntimeError(f"axon_start_nrt_profile rc={rc}")
        try:
            yield
        finally:
            n = lib.axon_stop_nrt_profile(str(output_dir).encode())
            print(f"profile: {n} file(s) written to {output_dir}")

    set_axon_ntff_profile_hook(_hook)


def kernel_traced(**inputs):
    """Like kernel() but runs with NTFF tracing; returns (out, exec_time_ns, tmpdir)."""
    import tempfile
    _install_ntff_hook()
    from concourse.bass_utils import run_bass_kernel_spmd

    # build in_maps the same way
    stimulus_set = np.asarray(inputs["stimulus_set"]) ; label_idx = np.asarray(inputs["label_idx"])
    embed = np.asarray(inputs["embed"], dtype=np.float32)
    key = (float(inputs["rho"]), float(inputs["temperature"]), float(inputs["lr_attention"]),
           float(inputs["lr_association"]), float(inputs["beta"]))
    if key not in _cache:
        _cache[key] = _build(*key)
    nc = _cache[key]
    embedB = embed.reshape(NCHUNK, P, D).transpose(1, 0, 2).reshape(P, NCHUNK * D).copy()
    z = embed[stimulus_set]
    onehot = np.zeros((B, T, U), dtype=np.float32)
    bi, ti = np.meshgrid(np.arange(B), np.arange(T), indexing="ij")
    onehot[bi, ti, label_idx] = 1.0
    in_maps = []
    for i in range(N_CORES):
        bs = slice(i * B_LOC, (i + 1) * B_LOC)
        zc = z[bs].transpose(1, 0, 2).reshape(1, T * B_LOC * D)
        zbcast = np.broadcast_to(zc, (P, T * B_LOC * D)).copy()
        ohrow = onehot[bs].transpose(1, 0, 2).reshape(1, T * B_LOC * U)
        ohfull = np.broadcast_to(ohrow, (P, T * B_LOC * U))
        big = np.concatenate([embedB, zbcast, ohfull], axis=1).astype(np.float32)
        in_maps.append({"big": np.ascontiguousarray(big)})
    tmpdir = tempfile.mkdtemp(prefix="alcove_trace_")
    res = run_bass_kernel_spmd(nc, in_maps, core_ids=list(range(N_CORES)), trace=True, tmpdir=tmpdir)
    outs = [res.results[i]["out"].reshape(B_LOC, T, U) for i in range(N_CORES)]
    return np.concatenate(outs, axis=0), res.exec_time_ns, tmpdir


# revision 17
# speedup vs baseline: 1.1175x; 1.1175x over previous
"""ALCOVE cell Bass kernel for 8 TRN2 NeuronCores (data-parallel over batch).

B=32, T=16, N_RBF=1024, N_DIM=64, UNITS=64. 4 batches per core.

Layout: R=1024 on partitions as 8 chunks of 128; per-batch row data
(att, x, dx, g_att) on partition 0 as (1, B_LOC*64) rows (PE base-partition
rule); partition broadcasts via K=1 ones-matmul. Big elementwise work runs
on (128, B_LOC*NCHUNK*64) = (128, 2048) tiles in single instructions;
contractions over the free dim use TT-mult + tensor_reduce; contractions
over partitions use M=1 accumulating matmuls.
"""

import numpy as np

B, T, R, D, U = 32, 16, 1024, 64, 64
NCHUNK, P = 8, 128
EPS = 1e-6
N_CORES = 8
B_LOC = B // N_CORES  # 4

_cache = {}


def _patch_act_tables():
    """Make every activation resolve to natural_log_exp_and_others (it
    contains abs/ln/exp/relu/copy/identity/square) so the kernel needs a
    single ACT table load instead of thrashing between sets."""
    import concourse.bacc as bacc_mod
    from concourse.hw_specs import get_activation_tables as _gat

    if getattr(bacc_mod.get_activation_tables, "_alcove_patched", False):
        return

    def patched(arch):
        t = _gat(arch)
        keep = t["natural_log_exp_and_others"]
        out = {}
        for name, fns in t.items():
            out[name] = fns if name == "natural_log_exp_and_others" else (fns - keep)
        return out

    patched._alcove_patched = True
    bacc_mod.get_activation_tables = patched


def _build(rho, temperature, lr_att, lr_assoc, beta):
    import concourse.bass as bass
    import concourse.tile as tile
    from concourse import bacc, mybir

    _patch_act_tables()

    f32 = mybir.dt.float32
    AF = mybir.ActivationFunctionType
    OP = mybir.AluOpType

    nc = bacc.Bacc("TRN2", target_bir_lowering=False, debug=False, num_devices=N_CORES)
    # single packed input: [embedB (512) | zbcast (T*B*D=4096) | oh row (T*B*U=1024)]
    FIN = NCHUNK * D + T * B_LOC * D + T * B_LOC * U
    big_in = nc.declare_dram_parameter("big", [P, FIN], f32, isOutput=False)
    out_ext = nc.declare_dram_parameter("out", [B_LOC, T * U], f32, isOutput=True)

    with tile.TileContext(nc) as tc:
        with (
            tc.tile_pool(name="persist", bufs=1) as persist,
            tc.tile_pool(name="work", bufs=3) as work,
            tc.tile_pool(name="psum", bufs=2, space="PSUM") as psum,
            tc.tile_pool(name="psmall", bufs=2, space="PSUM") as psmall,
        ):
            # ---- persistent tiles (one DMA for all inputs) ----
            big = persist.tile([P, FIN], f32)
            nc.gpsimd.dma_start(big[:], big_in[:])
            embedB = big[:, 0 : NCHUNK * D]
            zb = big[:, NCHUNK * D : NCHUNK * D + T * B_LOC * D].rearrange("p (t f) -> p t f", t=T)
            oh = big[0:1, NCHUNK * D + T * B_LOC * D :].rearrange("p (t f) -> p t f", t=T)

            ones_row = persist.tile([1, P], f32)
            nc.vector.memset(ones_row[:], 1.0)
            consts = persist.tile([P, 3], f32)
            nc.vector.memset(consts[:, 0:1], 0.0)
            nc.vector.memset(consts[:, 1:2], 1.0)
            nc.vector.memset(consts[:, 2:3], EPS)
            czero, cone, ceps = consts[:, 0:1], consts[:, 1:2], consts[:, 2:3]

            att = persist.tile([1, B_LOC * D], f32)  # row form, state
            nc.vector.memset(att[:], 1.0 / D)
            assoc = persist.tile([P, B_LOC, NCHUNK, U], f32)  # state
            nc.vector.memset(assoc[:], 0.0)

            xs4 = persist.tile([B_LOC, T, U], f32)
            probs = persist.tile([B_LOC, T, U], f32)
            sume = persist.tile([B_LOC, T], f32)

            # broadcast-view of embedB over batches: (P, B_LOC, NCHUNK, D)
            embed_bc = embedB.rearrange("p (c d) -> p c d", d=D)[:, None, :, :].broadcast_to([P, B_LOC, NCHUNK, D])

            for t in range(T):
                # -------- attention broadcast: (1, B*D) -> psum -> sbuf
                attb_ps = psum.tile([P, B_LOC, D], f32, tag="attb")
                nc.tensor.matmul(attb_ps[:, :, :].rearrange("p b d -> p (b d)"),
                                 ones_row[:], att[0:1, :], start=True, stop=True)
                attb = work.tile([P, B_LOC, D], f32, tag="attb_sb")
                nc.scalar.copy(attb[:], attb_ps[:])

                # -------- diff^rho chain on (P, B_LOC, NCHUNK, D) in one shot
                zrep = zb[:, t, :].rearrange("p (b d) -> p b d", d=D)[:, :, None, :].broadcast_to([P, B_LOC, NCHUNK, D])
                diff = work.tile([P, B_LOC, NCHUNK, D], f32, tag="diff")
                nc.vector.tensor_tensor(diff[:], embed_bc, zrep, op=OP.subtract)
                nc.scalar.activation(diff[:], diff[:], AF.Abs, bias=czero)
                nc.scalar.activation(diff[:], diff[:], AF.Ln, bias=ceps)
                dpow = work.tile([P, B_LOC, NCHUNK, D], f32, tag="dpow")
                nc.scalar.activation(dpow[:], diff[:], AF.Exp, bias=czero, scale=rho)

                # -------- q = sum_d att*dpow : TT mult + reduce
                qtmp = work.tile([P, B_LOC, NCHUNK, D], f32, tag="qtmp")
                attb_bc = attb[:, :, None, :].broadcast_to([P, B_LOC, NCHUNK, D])
                nc.vector.tensor_tensor(qtmp[:], dpow[:], attb_bc, op=OP.mult)
                qall = work.tile([P, B_LOC, NCHUNK], f32, tag="qall")
                nc.vector.tensor_reduce(qall[:], qtmp[:], axis=mybir.AxisListType.X, op=OP.add)

                # -------- similarity acts on (P, B_LOC*NCHUNK)
                lnq = work.tile([P, B_LOC, NCHUNK], f32, tag="lnq")
                nc.scalar.activation(lnq[:], qall[:], AF.Ln, bias=ceps)
                s_sim = work.tile([P, B_LOC, NCHUNK], f32, tag="s_sim")
                nc.scalar.activation(s_sim[:], lnq[:], AF.Exp, bias=czero, scale=1.0 / rho)  # = d
                nc.scalar.activation(s_sim[:], s_sim[:], AF.Exp, bias=czero, scale=-beta)    # = s
                qp = work.tile([P, B_LOC, NCHUNK], f32, tag="qp")
                nc.scalar.activation(qp[:], lnq[:], AF.Exp, bias=czero, scale=(1.0 - rho) / rho)

                # -------- x: per batch 8 accumulating matmuls, M=1 row 0
                x_ps = psmall.tile([1, B_LOC * U], f32, tag="x_ps")
                for b in range(B_LOC):
                    for c in range(NCHUNK):
                        nc.tensor.matmul(x_ps[0:1, b * U : (b + 1) * U],
                                         s_sim[:, b, c : c + 1],
                                         assoc[:, b, c, :],
                                         start=(c == 0), stop=(c == NCHUNK - 1))

                # -------- teacher / dx on (1, B_LOC*U)
                pp = work.tile([1, B_LOC * U], f32, tag="pp")
                nc.scalar.activation(pp[:], x_ps[:], AF.Relu, bias=cone[:1, :])          # relu(x+1)
                mrow = work.tile([1, B_LOC * U], f32, tag="mrow")
                nc.scalar.activation(mrow[:], x_ps[:], AF.Relu, bias=cone[:1, :], scale=-1.0)  # relu(1-x)
                # save x for softmax, scattered to (B_LOC, U)
                xrow = work.tile([1, B_LOC * U], f32, tag="xrow")
                nc.scalar.copy(xrow[:], x_ps[:])
                for b in range(B_LOC):
                    nc.gpsimd.dma_start(xs4[b : b + 1, t, :],
                                        xrow[0:1, b * U : (b + 1) * U])
                nc.vector.tensor_tensor(mrow[:], pp[:], mrow[:], op=OP.add)          # p+m
                nc.vector.tensor_tensor(mrow[:], mrow[:], oh[0:1, t, :], op=OP.mult) # oh*(p+m)
                dxrow = work.tile([1, B_LOC * U], f32, tag="dxrow")
                nc.vector.tensor_tensor(dxrow[:], pp[:], mrow[:], op=OP.subtract)

                # -------- dx broadcast to (P, B_LOC, U)
                dxb_ps = psum.tile([P, B_LOC, U], f32, tag="dxb")
                nc.tensor.matmul(dxb_ps[:, :, :].rearrange("p b d -> p (b d)"),
                                 ones_row[:], dxrow[0:1, :], start=True, stop=True)
                dxb = work.tile([P, B_LOC, U], f32, tag="dxb_sb")
                nc.scalar.copy(dxb[:], dxb_ps[:])
                dxb_bc = dxb[:, :, None, :].broadcast_to([P, B_LOC, NCHUNK, U])

                # -------- y = sum_u assoc*dx : TT mult + reduce
                ytmp = work.tile([P, B_LOC, NCHUNK, U], f32, tag="ytmp")
                nc.vector.tensor_tensor(ytmp[:], assoc[:], dxb_bc, op=OP.mult)
                yall = work.tile([P, B_LOC, NCHUNK], f32, tag="yall")
                nc.vector.tensor_reduce(yall[:], ytmp[:], axis=mybir.AxisListType.X, op=OP.add)

                # -------- c = -(beta/rho) * s * qp * y
                call = work.tile([P, B_LOC, NCHUNK], f32, tag="call")
                nc.vector.tensor_tensor(call[:], s_sim[:], qp[:], op=OP.mult)
                nc.vector.scalar_tensor_tensor(call[:], yall[:], -beta / rho, call[:],
                                               op0=OP.mult, op1=OP.mult)

                # -------- g_att (PE, M=1 row 0) + att update
                gatt_ps = psmall.tile([1, B_LOC * D], f32, tag="gatt")
                for b in range(B_LOC):
                    for c in range(NCHUNK):
                        nc.tensor.matmul(gatt_ps[0:1, b * D : (b + 1) * D],
                                         call[:, b, c : c + 1],
                                         dpow[:, b, c, :],
                                         start=(c == 0), stop=(c == NCHUNK - 1))
                nc.vector.scalar_tensor_tensor(att[:], gatt_ps[:], -lr_att, att[:],
                                               op0=OP.mult, op1=OP.add)
                nc.vector.tensor_scalar_max(att[:], att[:], 0.0)

                # -------- assoc update: assoc += (-lr*s_bc) * dx_bu (2 big TTs)
                slr = work.tile([P, B_LOC, NCHUNK], f32, tag="slr")
                nc.vector.tensor_scalar_mul(slr[:], s_sim[:], -lr_assoc)
                upd = work.tile([P, B_LOC, NCHUNK, U], f32, tag="upd")
                slr_bc = slr[:, :, :, None].broadcast_to([P, B_LOC, NCHUNK, U])
                nc.vector.tensor_tensor(upd[:], slr_bc, dxb_bc, op=OP.mult)
                nc.vector.tensor_tensor(assoc[:], assoc[:], upd[:], op=OP.add)

                # -------- softmax(temp*x) for this t (stable), (B_LOC, U)
                mx = work.tile([B_LOC, 1], f32, tag="mx")
                nc.vector.tensor_reduce(mx[:], xs4[:, t, :], axis=mybir.AxisListType.X, op=OP.max)
                nc.vector.tensor_scalar_mul(mx[:], mx[:], -temperature)
                nc.scalar.activation(probs[:, t, :], xs4[:, t, :], AF.Exp,
                                     bias=mx[:], scale=temperature,
                                     accum_out=sume[:, t : t + 1])

            # -------- normalize + store
            rec = work.tile([B_LOC, T], f32, tag="rec")
            nc.vector.reciprocal(rec[:], sume[:])
            for t in range(T):
                nc.vector.tensor_scalar_mul(probs[:, t, :], probs[:, t, :], rec[:, t : t + 1])
            nc.gpsimd.dma_start(out_ext[:], probs[:, :, :].rearrange("b t u -> b (t u)"))

    nc.compile()
    return nc


def _prep_in_maps(stimulus_set, label_idx, embed):
    embedB = embed.reshape(NCHUNK, P, D).transpose(1, 0, 2).reshape(P, NCHUNK * D)
    z = embed[stimulus_set]  # (B, T, D)
    onehot = np.zeros((B, T, U), dtype=np.float32)
    bi, ti = np.meshgrid(np.arange(B), np.arange(T), indexing="ij")
    onehot[bi, ti, label_idx] = 1.0
    in_maps = []
    for i in range(N_CORES):
        bs = slice(i * B_LOC, (i + 1) * B_LOC)
        zc = z[bs].transpose(1, 0, 2).reshape(1, T * B_LOC * D)
        zbcast = np.broadcast_to(zc, (P, T * B_LOC * D))
        ohrow = onehot[bs].transpose(1, 0, 2).reshape(1, T * B_LOC * U)
        ohfull = np.broadcast_to(ohrow, (P, T * B_LOC * U))
        big = np.concatenate([embedB, zbcast, ohfull], axis=1).astype(np.float32)
        in_maps.append({"big": np.ascontiguousarray(big)})
    return in_maps


def kernel(stimulus_set, label_idx, embed, rho, temperature, lr_attention, lr_association, beta):
    from concourse.bass_utils import run_bass_kernel_spmd

    stimulus_set = np.asarray(stimulus_set)
    label_idx = np.asarray(label_idx)
    embed = np.asarray(embed, dtype=np.float32)
    key = (float(rho), float(temperature), float(lr_attention),
           float(lr_association), float(beta))
    if key not in _cache:
        _cache[key] = _build(*key)
    nc = _cache[key]
    in_maps = _prep_in_maps(stimulus_set, label_idx, embed)
    res = run_bass_kernel_spmd(nc, in_maps, core_ids=list(range(N_CORES)))
    outs = [res.results[i]["out"].reshape(B_LOC, T, U) for i in range(N_CORES)]
    return np.concatenate(outs, axis=0)


def _install_ntff_hook():
    import sys, types, ctypes, contextlib
    if "antenv.axon_hooks" in sys.modules:
        return
    import antenv
    mod = types.ModuleType("antenv.axon_hooks")
    mod._hook = None
    def set_axon_ntff_profile_hook(h):
        mod._hook = h
    def get_axon_ntff_profile_hook():
        return mod._hook
    mod.set_axon_ntff_profile_hook = set_axon_ntff_profile_hook
    mod.get_axon_ntff_profile_hook = get_axon_ntff_profile_hook
    sys.modules["antenv.axon_hooks"] = mod
    antenv.axon_hooks = mod

    lib = ctypes.CDLL("/opt/axon/libaxon_pjrt.so")
    if not hasattr(lib, "axon_start_nrt_profile"):
        return
    lib.axon_start_nrt_profile.argtypes = [ctypes.POINTER(ctypes.c_int64), ctypes.c_size_t]
    lib.axon_start_nrt_profile.restype = ctypes.c_int64
    lib.axon_stop_nrt_profile.argtypes = [ctypes.c_char_p]
    lib.axon_stop_nrt_profile.restype = ctypes.c_int64

    @contextlib.contextmanager
    def _hook(output_dir, device_ids):
        import jax
        jax.devices()
        if device_ids:
            ids = (ctypes.c_int64 * len(device_ids))(*device_ids)
            rc = lib.axon_start_nrt_profile(ids, len(device_ids))
        else:
            rc = lib.axon_start_nrt_profile(None, 0)
        if rc != 0:
            raise RuntimeError(f"axon_start_nrt_profile rc={rc}")
        try:
            yield
        finally:
            n = lib.axon_stop_nrt_profile(str(output_dir).encode())
            print(f"profile: {n} file(s) written to {output_dir}")

    set_axon_ntff_profile_hook(_hook)


def kernel_traced(**inputs):
    """Like kernel() but runs with NTFF tracing; returns (out, exec_time_ns, tmpdir)."""
    import tempfile
    _install_ntff_hook()
    from concourse.bass_utils import run_bass_kernel_spmd

    key = (float(inputs["rho"]), float(inputs["temperature"]), float(inputs["lr_attention"]),
           float(inputs["lr_association"]), float(inputs["beta"]))
    if key not in _cache:
        _cache[key] = _build(*key)
    nc = _cache[key]
    in_maps = _prep_in_maps(np.asarray(inputs["stimulus_set"]), np.asarray(inputs["label_idx"]),
                            np.asarray(inputs["embed"], dtype=np.float32))
    tmpdir = tempfile.mkdtemp(prefix="alcove_trace_")
    res = run_bass_kernel_spmd(nc, in_maps, core_ids=list(range(N_CORES)), trace=True, tmpdir=tmpdir)
    outs = [res.results[i]["out"].reshape(B_LOC, T, U) for i in range(N_CORES)]
    return np.concatenate(outs, axis=0), res.exec_time_ns, tmpdir
